# revision 42
# baseline (speedup 1.0000x reference)
"""Block-FFT circulant matmul (BlockFFTDirectPrior) as a Trainium2 Bass kernel.

Math: out = ifft( einsum('bjf,ijf->bif', fft(x_blocks), conj(W_full)) ).real
with 64x64 blocks of size 256, batch 2048.

Everything is real-matmul based (no complex arithmetic, no FFT butterflies):
  stage 1: per input block j, spectrum = x_j @ R            (DFT as matmul)
  stage 2: per frequency slot s, mix blocks j -> i with a 128x128 real
           matrix G_s built from W_real/W_imag (re/im packed)
  stage 3: per output block i, time = spectrum_i @ Rinv     (IDFT as matmul)

Spectrum packing (256 real values per block): half A = Re f=0..127,
half B = [Re f=128, Im f=1..127].  Slot s pairs (A[s], B[s]): slot 0
carries DC/Nyquist (both real), slots 1..127 carry complex bin f=s.

Between stages the partition axis must rotate (spec -> block -> spec).
Both permutes are done with the DMA-transpose XBAR (InstDmaTransposeAnt):
one instruction transposes [P, (A-major, B-minor)] -> [B, (A-major, P-minor)]
at ~14ns per 16x128 source tile, entirely off the PE/DVE/Act engines.

Sharding: data-parallel over batch across 8 NeuronCores (256 rows each),
2 passes of 128 rows per core.  All matmul operands are bf16 (PSUM
accumulation is fp32); output is stored bf16 and upcast on host.
"""

import numpy as np
import ml_dtypes

import concourse.bass as bass
import concourse.mybir as mybir
from concourse import bacc
from concourse.tile import TileContext
from concourse.bass_utils import run_bass_kernel_spmd

B, KIN, KOUT, BLOCK = 2048, 64, 64, 256
NCORES = 8
BC = B // NCORES            # 256 batch rows per core
NPASS = 2
PB = BC // NPASS            # 128 batch rows per pass

F32 = mybir.dt.float32
BF16 = mybir.dt.bfloat16
NPBF16 = ml_dtypes.bfloat16

_NC_CACHE = {}


def _build_consts():
    """DFT / inverse-DFT matrices, bf16, kernel layouts."""
    t = np.arange(BLOCK)
    f = np.arange(128)
    ang = 2.0 * np.pi * np.outer(t, f) / BLOCK          # [t, f]
    RA = np.cos(ang)                                    # re f=0..127
    RB = -np.sin(ang)                                   # im f=1..127
    RB[:, 0] = np.cos(np.pi * t)                        # re f=128 in col 0
    R = np.zeros((2, 2, 128, 128), dtype=NPBF16)        # [h, kt, t(128), m]
    for kt in range(2):
        R[0, kt] = RA[kt * 128:(kt + 1) * 128, :].astype(NPBF16)
        R[1, kt] = RB[kt * 128:(kt + 1) * 128, :].astype(NPBF16)

    s = np.arange(128)
    tp = np.arange(BLOCK)
    angi = 2.0 * np.pi * np.outer(s, tp) / BLOCK        # [s, t']
    w = np.full((128, 1), 2.0 / BLOCK)
    w[0] = 1.0 / BLOCK
    RiA = w * np.cos(angi)
    RiB = -(2.0 / BLOCK) * np.sin(angi)
    RiB[0, :] = (1.0 / BLOCK) * np.cos(np.pi * tp)      # Nyquist (real) term
    Ri = np.stack([RiA, RiB]).astype(NPBF16)            # [2, 128, 256]
    return R, Ri


def _build_g(Wr, Wi):
    """Stage-2 mixing matrices, layout [k=(h*64+j), s, m=(re_i|im_i)], bf16."""
    G = np.zeros((128, 128, 128), dtype=np.float32)     # [s, k, m]
    G[0, :64, :64] = Wr[:, :, 0].T
    G[0, 64:, 64:] = Wr[:, :, 128].T
    WrT = np.transpose(Wr, (2, 1, 0))                   # [f, j, i]
    WiT = np.transpose(Wi, (2, 1, 0))
    G[1:, :64, :64] = WrT[1:128]
    G[1:, :64, 64:] = -WiT[1:128]
    G[1:, 64:, :64] = WiT[1:128]
    G[1:, 64:, 64:] = WrT[1:128]
    return np.ascontiguousarray(G.transpose(1, 0, 2)).astype(NPBF16)


def _build_nc():
    nc = bacc.Bacc("TRN2", target_bir_lowering=False, debug=False)
    # xP layout [pass, t(256), b(128), j(64)]
    xP = nc.dram_tensor("xP", [NPASS, BLOCK, PB, KIN], BF16, kind="ExternalInput")
    Gt = nc.dram_tensor("G", [128, 128, 128], BF16, kind="ExternalInput")
    Rt = nc.dram_tensor("R", [2, 2, 128, 128], BF16, kind="ExternalInput")
    Rit = nc.dram_tensor("Ri", [2, 128, 256], BF16, kind="ExternalInput")
    Y = nc.dram_tensor("Y", [BC, KOUT * BLOCK], BF16, kind="ExternalOutput")

    def copy_eng(k):
        return nc.vector.tensor_copy if k % 2 == 0 else nc.scalar.copy

    with TileContext(nc) as tc:
        with (
            tc.tile_pool(name="const", bufs=1) as cpool,
            tc.tile_pool(name="big", bufs=4) as bigpool,
            tc.tile_pool(name="xk", bufs=1) as xkpool,
            tc.tile_pool(name="yt", bufs=2) as ytpool,
            tc.tile_pool(name="ps", bufs=4, space="PSUM") as pspool,
        ):
            # ---- constants (Act hwdge queue) ----
            Rsb = cpool.tile([128, 4 * 128], BF16)
            for h in range(2):
                for kt in range(2):
                    nc.scalar.dma_start(
                        Rsb[:, (h * 2 + kt) * 128:(h * 2 + kt + 1) * 128],
                        Rt.ap()[h, kt],
                    )
            Risb = cpool.tile([128, 512], BF16)
            for h in range(2):
                nc.scalar.dma_start(Risb[:, h * 256:(h + 1) * 256], Rit.ap()[h])

            # big 32KB/partition tiles, 4-slot rotation (see request order
            # below: out1p1, X2p1, out1p2, X2p2, O2p1, T2p1, O2p2, T2p2)
            def big(name):
                return bigpool.tile([128, 16384], BF16, tag="big", name=name)

            nck = 0

            # ---------------- stage 1: DFT per block ----------------
            # xkc[kt][c] [t-half, (b32, j64)]; out1 [s, (b, hj)]
            def load_xk_chunk(p, c, xkc):
                for kt in range(2):
                    xt = xkpool.tile([128, 2048], BF16, tag=f"xk{kt}c{c}",
                                     name=f"xk{kt}c{c}p{p}")
                    nc.scalar.dma_start(
                        xt[:, :],
                        xP.ap()[p, kt * 128:(kt + 1) * 128,
                                c * 32:(c + 1) * 32],
                    )
                    xkc[kt][c] = xt

            def load_xk(p):
                # c-outer, kt-inner so the first (kt0,kt1) pair lands first
                # and stage-1 can start after ~2 chunk loads.
                xkc = [[None] * 4 for _ in range(2)]
                for c in range(4):
                    load_xk_chunk(p, c, xkc)
                return xkc

            def stage1(p, xkc, out1, x2, prefetch=None):
                # prefetch = (next_pass, xkc_next): emit the next pass's
                # chunk-c load right after this pass frees chunk c, so the
                # Act-queue dispatch never head-of-line blocks the copies
                # queued behind it.
                nonlocal nck
                out1v = out1.rearrange("p (b hj) -> p b hj", hj=128)
                x2v = x2.rearrange("p (b s) -> p b s", s=128)
                for g2 in range(8):      # b 16-group
                    for h in range(2):
                        ps1 = pspool.tile([128, 1024], F32, tag="ps")
                        for q in range(2):
                            for kt in range(2):
                                g = g2 * 2 + q
                                c, loc = g // 4, (g % 4) * 512
                                nc.tensor.matmul(
                                    ps1[:, q * 512:(q + 1) * 512],
                                    Rsb[:, (h * 2 + kt) * 128:
                                        (h * 2 + kt + 1) * 128],
                                    xkc[kt][c][:, loc:loc + 512],
                                    start=(kt == 0), stop=(kt == 1),
                                )
                        # ps1 [s, (b16, j64)] -> out1 [s, b, h*64+j]
                        copy_eng(nck)(
                            out1v[:, g2 * 16:(g2 + 1) * 16, h * 64:(h + 1) * 64],
                            ps1.rearrange("p (b j) -> p b j", b=16),
                        )
                        nck += 1
                    if g2 % 2 == 1:  # b 32-chunk complete -> xbar chunk
                        b0 = (g2 // 2) * 32
                        nc.sync.dma_start(
                            x2v[:, b0:b0 + 32],
                            out1[:, b0 * 128:(b0 + 32) * 128],
                            transpose=True,
                        )
                        if prefetch is not None:
                            load_xk_chunk(prefetch[0], g2 // 2, prefetch[1])

            # ---------------- stage 2: mix blocks per slot -----------
            # X2 [hj, (b, s)]; O2 [m, (b, s)]; Gsb [hj, (s, m)] resident
            def stage2(p, x2, o2, Gsb):
                nonlocal nck
                x2v = x2.rearrange("p (b s) -> p s b", s=128)
                o2v = o2.rearrange("p (b s) -> p b s", s=128)
                for g8 in range(16):     # 8 slots per PSUM tile
                    ps2 = pspool.tile([128, 1024], F32, tag="ps")
                    for q in range(8):
                        s = 8 * g8 + q
                        nc.tensor.matmul(
                            ps2[:, q * 128:(q + 1) * 128],
                            Gsb[:, s * 128:(s + 1) * 128],
                            x2v[:, s, :],
                            start=True, stop=True,
                        )
                    # ps2 [m, (s8, b)] -> O2 [m, b, s0:s0+8]
                    copy_eng(nck)(
                        o2v[:, :, g8 * 8:(g8 + 1) * 8],
                        ps2.rearrange("p (s b) -> p b s", s=8),
                    )
                    nck += 1

            # ---------- stage 3: IDFT per output block ---------------
            # T2 [s, (b, m)]
            def stage3(p, t2):
                nonlocal nck
                t2v = t2.rearrange("p (b m) -> p m b", m=128)
                for g8 in range(8):      # 8 output blocks i per store
                    yt = ytpool.tile([128, 2048], BF16, tag="yt",
                                     name=f"yt{g8}p{p}")
                    for half in range(2):    # 4 blocks i per PSUM tile
                        ps3 = pspool.tile([128, 1024], F32, tag="ps")
                        for q in range(4):
                            i = g8 * 8 + half * 4 + q
                            nc.tensor.matmul(
                                ps3[:, q * 256:(q + 1) * 256],
                                t2v[:, i, :],
                                Risb[:, 0:256], start=True, stop=False,
                            )
                            nc.tensor.matmul(
                                ps3[:, q * 256:(q + 1) * 256],
                                t2v[:, 64 + i, :],
                                Risb[:, 256:512], start=False, stop=True,
                            )
                        copy_eng(nck)(
                            yt[:, half * 1024:(half + 1) * 1024], ps3[:, :]
                        )
                        nck += 1
                    nc.sync.dma_start(
                        Y.ap()[p * PB:(p + 1) * PB, g8 * 2048:(g8 + 1) * 2048],
                        yt[:, :],
                    )

            def perm2(o2, t2, deprio=False):
                # O2 [m, (b, s)] -> T2 [s, (b, m)], two b-half xbar chunks.
                # Both chunks dispatched from the sync queue.  deprio pushes
                # the chunks later in the scheduler's modeled order so other
                # instructions' completion-counter thresholds (which the
                # scheduler coalesces to its modeled completion times) do
                # not transitively include them.
                t2v = t2.rearrange("p (b m) -> p b m", m=128)
                for half in range(2):
                    b0 = half * 64
                    if deprio:
                        with tc.high_priority(offset=-1000000):
                            nc.sync.dma_start(
                                t2v[:, b0:b0 + 64],
                                o2[:, b0 * 128:(b0 + 64) * 128],
                                transpose=True,
                            )
                    else:
                        nc.sync.dma_start(
                            t2v[:, b0:b0 + 64],
                            o2[:, b0 * 128:(b0 + 64) * 128],
                            transpose=True,
                        )

            # ---- emission. Queue roles:
            #   Act (scalar): consts, x loads (both passes), G, half the
            #                 PSUM->SBUF copies
            #   sync (SP):    all xbar transposes + Y stores
            #   DVE (vector): the other half of the copies
            xk1 = load_xk(0)
            # Deprioritize the (dependency-free) G load so the scheduler
            # does not hoist it ahead of the pass-2 x loads in its modeled
            # order — that inflates the Act-queue completion thresholds the
            # pass-2 stage-1 matmuls wait on.
            Gsb = cpool.tile([128, 16384], BF16, name="Gsb")
            with tc.high_priority(offset=-1000000):
                nc.scalar.dma_start(Gsb[:, :], Gt.ap())

            out1p1 = big("out1p1")
            x2p1 = big("x2p1")
            xk2 = [[None] * 4 for _ in range(2)]
            stage1(0, xk1, out1p1, x2p1, prefetch=(1, xk2))

            out1p2 = big("out1p2")
            x2p2 = big("x2p2")
            stage1(1, xk2, out1p2, x2p2)

            o2p1 = big("o2p1")
            stage2(0, x2p1, o2p1, Gsb)
            t2p1 = big("t2p1")
            perm2(o2p1, t2p1)

            o2p2 = big("o2p2")
            stage2(1, x2p2, o2p2, Gsb)
            t2p2 = big("t2p2")
            perm2(o2p2, t2p2, deprio=True)

            stage3(0, t2p1)
            stage3(1, t2p2)

    nc.compile()
    return nc


def _get_nc():
    if "nc" not in _NC_CACHE:
        _NC_CACHE["nc"] = _build_nc()
    return _NC_CACHE["nc"]


def run(x, W_real, W_imag, trace=False):
    x = np.asarray(x, dtype=np.float32)
    Wr = np.asarray(W_real, dtype=np.float32)
    Wi = np.asarray(W_imag, dtype=np.float32)

    nc = _get_nc()
    R, Ri = _build_consts()
    G = _build_g(Wr, Wi)
    in_maps = []
    for c in range(NCORES):
        xc = x[c * BC:(c + 1) * BC]                       # [256, 16384]
        # -> [t, b, j] -> [pass, t(256), b(128), j(64)]
        xcp = xc.reshape(BC, KIN, BLOCK).transpose(2, 0, 1)
        xcp = xcp.reshape(BLOCK, NPASS, PB, KIN).transpose(1, 0, 2, 3)
        in_maps.append({
            "xP": np.ascontiguousarray(xcp).astype(NPBF16),
            "G": G, "R": R, "Ri": Ri,
        })
    res = run_bass_kernel_spmd(
        nc, in_maps, core_ids=list(range(NCORES)), trace=trace
    )
    out = np.concatenate([r["Y"] for r in res.results], axis=0)
    return np.ascontiguousarray(out).astype(np.float32), res


def kernel(x, W_real, W_imag):
    out, _ = run(x, W_real, W_imag)
    return out


# revision 46
# speedup vs baseline: 1.0841x; 1.0841x over previous
"""Block-FFT circulant matmul (BlockFFTDirectPrior) as a Trainium2 Bass kernel.

Math: out = ifft( einsum('bjf,ijf->bif', fft(x_blocks), conj(W_full)) ).real
with 64x64 blocks of size 256, batch 2048.

Everything is real-matmul based (no complex arithmetic, no FFT butterflies):
  stage 1: per input block j, spectrum = x_j @ R            (DFT as matmul)
  stage 2: per frequency slot s, mix blocks j -> i with a 128x128 real
           matrix G_s built from W_real/W_imag (re/im packed)
  stage 3: per output block i, time = spectrum_i @ Rinv     (IDFT as matmul)

Spectrum packing (256 real values per block): half A = Re f=0..127,
half B = [Re f=128, Im f=1..127].  Slot s pairs (A[s], B[s]): slot 0
carries DC/Nyquist (both real), slots 1..127 carry complex bin f=s.

Between stages the partition axis must rotate (spec -> block -> spec).
Both permutes are done with the DMA-transpose XBAR (InstDmaTransposeAnt):
one instruction transposes [P, (A-major, B-minor)] -> [B, (A-major, P-minor)]
at ~14ns per 16x128 source tile, entirely off the PE/DVE/Act engines.

Sharding: data-parallel over batch across 8 NeuronCores (256 rows each),
2 passes of 128 rows per core.  All matmul operands are bf16 (PSUM
accumulation is fp32); output is stored bf16 and upcast on host.
"""

import numpy as np
import ml_dtypes

import concourse.bass as bass
import concourse.mybir as mybir
from concourse import bacc
from concourse.tile import TileContext
from concourse.bass_utils import run_bass_kernel_spmd

B, KIN, KOUT, BLOCK = 2048, 64, 64, 256
NCORES = 8
BC = B // NCORES            # 256 batch rows per core
NPASS = 2
PB = BC // NPASS            # 128 batch rows per pass

F32 = mybir.dt.float32
BF16 = mybir.dt.bfloat16
NPBF16 = ml_dtypes.bfloat16

_NC_CACHE = {}


def _build_consts():
    """DFT / inverse-DFT matrices, bf16, kernel layouts."""
    t = np.arange(BLOCK)
    f = np.arange(128)
    ang = 2.0 * np.pi * np.outer(t, f) / BLOCK          # [t, f]
    RA = np.cos(ang)                                    # re f=0..127
    RB = -np.sin(ang)                                   # im f=1..127
    RB[:, 0] = np.cos(np.pi * t)                        # re f=128 in col 0
    R = np.zeros((2, 2, 128, 128), dtype=NPBF16)        # [h, kt, t(128), m]
    for kt in range(2):
        R[0, kt] = RA[kt * 128:(kt + 1) * 128, :].astype(NPBF16)
        R[1, kt] = RB[kt * 128:(kt + 1) * 128, :].astype(NPBF16)

    s = np.arange(128)
    tp = np.arange(BLOCK)
    angi = 2.0 * np.pi * np.outer(s, tp) / BLOCK        # [s, t']
    w = np.full((128, 1), 2.0 / BLOCK)
    w[0] = 1.0 / BLOCK
    RiA = w * np.cos(angi)
    RiB = -(2.0 / BLOCK) * np.sin(angi)
    RiB[0, :] = (1.0 / BLOCK) * np.cos(np.pi * tp)      # Nyquist (real) term
    Ri = np.stack([RiA, RiB]).astype(NPBF16)            # [2, 128, 256]
    return R, Ri


def _build_g(Wr, Wi):
    """Stage-2 mixing matrices, layout [k=(h*64+j), s, m=(re_i|im_i)], bf16."""
    G = np.zeros((128, 128, 128), dtype=np.float32)     # [s, k, m]
    G[0, :64, :64] = Wr[:, :, 0].T
    G[0, 64:, 64:] = Wr[:, :, 128].T
    WrT = np.transpose(Wr, (2, 1, 0))                   # [f, j, i]
    WiT = np.transpose(Wi, (2, 1, 0))
    G[1:, :64, :64] = WrT[1:128]
    G[1:, :64, 64:] = -WiT[1:128]
    G[1:, 64:, :64] = WiT[1:128]
    G[1:, 64:, 64:] = WrT[1:128]
    return np.ascontiguousarray(G.transpose(1, 0, 2)).astype(NPBF16)


def _build_nc():
    nc = bacc.Bacc("TRN2", target_bir_lowering=False, debug=False)
    # xP layout [pass, t(256), b(128), j(64)]
    xP = nc.dram_tensor("xP", [NPASS, BLOCK, PB, KIN], BF16, kind="ExternalInput")
    Gt = nc.dram_tensor("G", [128, 128, 128], BF16, kind="ExternalInput")
    Rt = nc.dram_tensor("R", [2, 2, 128, 128], BF16, kind="ExternalInput")
    Rit = nc.dram_tensor("Ri", [2, 128, 256], BF16, kind="ExternalInput")
    Y = nc.dram_tensor("Y", [BC, KOUT * BLOCK], BF16, kind="ExternalOutput")

    def copy_eng(k):
        return nc.vector.tensor_copy if k % 2 == 0 else nc.scalar.copy

    with TileContext(nc) as tc:
        with (
            tc.tile_pool(name="const", bufs=1) as cpool,
            tc.tile_pool(name="big", bufs=4) as bigpool,
            tc.tile_pool(name="xk", bufs=1) as xkpool,
            tc.tile_pool(name="yt", bufs=2) as ytpool,
            tc.tile_pool(name="ps", bufs=4, space="PSUM") as pspool,
        ):
            # ---- constants (Act hwdge queue) ----
            Rsb = cpool.tile([128, 4 * 128], BF16)
            for h in range(2):
                for kt in range(2):
                    nc.scalar.dma_start(
                        Rsb[:, (h * 2 + kt) * 128:(h * 2 + kt + 1) * 128],
                        Rt.ap()[h, kt],
                    )
            Risb = cpool.tile([128, 512], BF16)
            for h in range(2):
                nc.scalar.dma_start(Risb[:, h * 256:(h + 1) * 256], Rit.ap()[h])

            # big 32KB/partition tiles, 4-slot rotation (see request order
            # below: out1p1, X2p1, out1p2, X2p2, O2p1, T2p1, O2p2, T2p2)
            def big(name):
                return bigpool.tile([128, 16384], BF16, tag="big", name=name)

            nck = 0

            # ---------------- stage 1: DFT per block ----------------
            # xkc[kt][c] [t-half, (b32, j64)]; out1 [s, (b, hj)]
            def load_xk_chunk(p, c, xkc):
                for kt in range(2):
                    xt = xkpool.tile([128, 2048], BF16, tag=f"xk{kt}c{c}",
                                     name=f"xk{kt}c{c}p{p}")
                    nc.scalar.dma_start(
                        xt[:, :],
                        xP.ap()[p, kt * 128:(kt + 1) * 128,
                                c * 32:(c + 1) * 32],
                    )
                    xkc[kt][c] = xt

            def load_xk(p):
                # c-outer, kt-inner so the first (kt0,kt1) pair lands first
                # and stage-1 can start after ~2 chunk loads.
                xkc = [[None] * 4 for _ in range(2)]
                for c in range(4):
                    load_xk_chunk(p, c, xkc)
                return xkc

            def stage1(p, xkc, out1, x2):
                nonlocal nck
                out1v = out1.rearrange("p (b hj) -> p b hj", hj=128)
                x2v = x2.rearrange("p (b s) -> p b s", s=128)
                for g2 in range(8):      # b 16-group
                    for h in range(2):
                        ps1 = pspool.tile([128, 1024], F32, tag="ps")
                        for q in range(2):
                            for kt in range(2):
                                g = g2 * 2 + q
                                c, loc = g // 4, (g % 4) * 512
                                nc.tensor.matmul(
                                    ps1[:, q * 512:(q + 1) * 512],
                                    Rsb[:, (h * 2 + kt) * 128:
                                        (h * 2 + kt + 1) * 128],
                                    xkc[kt][c][:, loc:loc + 512],
                                    start=(kt == 0), stop=(kt == 1),
                                )
                        # ps1 [s, (b16, j64)] -> out1 [s, b, h*64+j].
                        # DVE only: keeps stage-1 copies off the Act queue,
                        # whose completion counter is held back by the
                        # head-of-line-blocked pass-2 x-load dispatches (the
                        # perm1 xbars wait on these copies).
                        nc.vector.tensor_copy(
                            out1v[:, g2 * 16:(g2 + 1) * 16, h * 64:(h + 1) * 64],
                            ps1.rearrange("p (b j) -> p b j", b=16),
                        )
                    if g2 % 2 == 1:  # b 32-chunk complete -> xbar chunk
                        b0 = (g2 // 2) * 32
                        nc.sync.dma_start(
                            x2v[:, b0:b0 + 32],
                            out1[:, b0 * 128:(b0 + 32) * 128],
                            transpose=True,
                        )

            # ---------------- stage 2: mix blocks per slot -----------
            # X2 [hj, (b, s)]; O2 [m, (b, s)]; Gsb [hj, (s, m)] resident
            def stage2(p, x2, o2, Gsb):
                nonlocal nck
                x2v = x2.rearrange("p (b s) -> p s b", s=128)
                o2v = o2.rearrange("p (b s) -> p b s", s=128)
                for g8 in range(16):     # 8 slots per PSUM tile
                    ps2 = pspool.tile([128, 1024], F32, tag="ps")
                    for q in range(8):
                        s = 8 * g8 + q
                        nc.tensor.matmul(
                            ps2[:, q * 128:(q + 1) * 128],
                            Gsb[:, s * 128:(s + 1) * 128],
                            x2v[:, s, :],
                            start=True, stop=True,
                        )
                    # ps2 [m, (s8, b)] -> O2 [m, b, s0:s0+8]
                    copy_eng(nck)(
                        o2v[:, :, g8 * 8:(g8 + 1) * 8],
                        ps2.rearrange("p (s b) -> p b s", s=8),
                    )
                    nck += 1

            # ---------- stage 3: IDFT per output block ---------------
            # T2 [s, (b, m)]
            def stage3(p, t2):
                nonlocal nck
                t2v = t2.rearrange("p (b m) -> p m b", m=128)
                for g8 in range(8):      # 8 output blocks i per store
                    yt = ytpool.tile([128, 2048], BF16, tag="yt",
                                     name=f"yt{g8}p{p}")
                    for half in range(2):    # 4 blocks i per PSUM tile
                        ps3 = pspool.tile([128, 1024], F32, tag="ps")
                        for q in range(4):
                            i = g8 * 8 + half * 4 + q
                            nc.tensor.matmul(
                                ps3[:, q * 256:(q + 1) * 256],
                                t2v[:, i, :],
                                Risb[:, 0:256], start=True, stop=False,
                            )
                            nc.tensor.matmul(
                                ps3[:, q * 256:(q + 1) * 256],
                                t2v[:, 64 + i, :],
                                Risb[:, 256:512], start=False, stop=True,
                            )
                        copy_eng(nck)(
                            yt[:, half * 1024:(half + 1) * 1024], ps3[:, :]
                        )
                        nck += 1
                    nc.sync.dma_start(
                        Y.ap()[p * PB:(p + 1) * PB, g8 * 2048:(g8 + 1) * 2048],
                        yt[:, :],
                    )

            def perm2(o2, t2, deprio=False):
                # O2 [m, (b, s)] -> T2 [s, (b, m)], two b-half xbar chunks.
                # Both chunks dispatched from the sync queue.  deprio pushes
                # the chunks later in the scheduler's modeled order so other
                # instructions' completion-counter thresholds (which the
                # scheduler coalesces to its modeled completion times) do
                # not transitively include them.
                t2v = t2.rearrange("p (b m) -> p b m", m=128)
                for half in range(2):
                    b0 = half * 64
                    if deprio:
                        with tc.high_priority(offset=-1000000):
                            nc.sync.dma_start(
                                t2v[:, b0:b0 + 64],
                                o2[:, b0 * 128:(b0 + 64) * 128],
                                transpose=True,
                            )
                    else:
                        nc.sync.dma_start(
                            t2v[:, b0:b0 + 64],
                            o2[:, b0 * 128:(b0 + 64) * 128],
                            transpose=True,
                        )

            # ---- emission. Queue roles:
            #   Act (scalar): consts, x loads (both passes), G, half the
            #                 PSUM->SBUF copies
            #   sync (SP):    all xbar transposes + Y stores
            #   DVE (vector): the other half of the copies
            xk1 = load_xk(0)
            xk2 = load_xk(1)
            # Deprioritize the (dependency-free) G load so the scheduler
            # does not hoist it ahead of the pass-2 x loads in its modeled
            # order — that inflates the Act-queue completion thresholds the
            # pass-2 stage-1 matmuls wait on.
            Gsb = cpool.tile([128, 16384], BF16, name="Gsb")
            with tc.high_priority(offset=-1000000):
                nc.scalar.dma_start(Gsb[:, :], Gt.ap())

            out1p1 = big("out1p1")
            x2p1 = big("x2p1")
            stage1(0, xk1, out1p1, x2p1)

            out1p2 = big("out1p2")
            x2p2 = big("x2p2")
            stage1(1, xk2, out1p2, x2p2)

            o2p1 = big("o2p1")
            stage2(0, x2p1, o2p1, Gsb)
            t2p1 = big("t2p1")
            perm2(o2p1, t2p1)

            o2p2 = big("o2p2")
            stage2(1, x2p2, o2p2, Gsb)
            t2p2 = big("t2p2")
            perm2(o2p2, t2p2, deprio=True)

            stage3(0, t2p1)
            stage3(1, t2p2)

    nc.compile()
    return nc


def _get_nc():
    if "nc" not in _NC_CACHE:
        _NC_CACHE["nc"] = _build_nc()
    return _NC_CACHE["nc"]


def run(x, W_real, W_imag, trace=False):
    x = np.asarray(x, dtype=np.float32)
    Wr = np.asarray(W_real, dtype=np.float32)
    Wi = np.asarray(W_imag, dtype=np.float32)

    nc = _get_nc()
    R, Ri = _build_consts()
    G = _build_g(Wr, Wi)
    in_maps = []
    for c in range(NCORES):
        xc = x[c * BC:(c + 1) * BC]                       # [256, 16384]
        # -> [t, b, j] -> [pass, t(256), b(128), j(64)]
        xcp = xc.reshape(BC, KIN, BLOCK).transpose(2, 0, 1)
        xcp = xcp.reshape(BLOCK, NPASS, PB, KIN).transpose(1, 0, 2, 3)
        in_maps.append({
            "xP": np.ascontiguousarray(xcp).astype(NPBF16),
            "G": G, "R": R, "Ri": Ri,
        })
    res = run_bass_kernel_spmd(
        nc, in_maps, core_ids=list(range(NCORES)), trace=trace
    )
    out = np.concatenate([r["Y"] for r in res.results], axis=0)
    return np.ascontiguousarray(out).astype(np.float32), res


def kernel(x, W_real, W_imag):
    out, _ = run(x, W_real, W_imag)
    return out
